# revision 50
# baseline (speedup 1.0000x reference)
"""Multi-head causal attention (B=4, S=2048, D=1024, H=16) on 8 TRN2 cores.

Sharding: core c -> batch c//2, head-group c%2 (8 heads, 512 of the 1024
QKV columns / Wo rows).  Each core runs a fused QKV->attention->out-proj
kernel on its shard; the host sums the two head-group partials per batch.

Per-core layout choices:
  - x is fed pre-transposed (xT [D, S]) so Q^T/K^T come out of the PE in
    [m, s] layout and V in natural [s, m] layout with no on-chip transposes.
  - scores are computed transposed (S^T [k, q]); softmax runs as
    exp (ScalarE, scale=1/8 fused, both heads of a pair in one op) ->
    causal mask (gpsimd affine_select, fill=0, diagonal tiles only,
    fully-masked q-ranges skipped entirely) -> attnV matmul with a
    ones-column appended to V (M=65) so the softmax denominator
    accumulates for free in PSUM row 64.
  - attnV trails exp by TWO kt, so each exp has the window of two scores
    matmuls plus fillers before the PE needs its output or its PSUM slot.
  - exp waterfall: each chunk's first PRE off-diagonal (t, kt) tiles are
    scored+exp'd a chunk EARLY (as fillers, after that chunk's QKV) with
    the u tiles parked in SBUF — the last chunk is exp-bound on ScalarE
    while the first has ScalarE slack.  The early-chunk denominator-row
    and cn drains likewise run on ScalarE (DVE is the binding engine
    there), switching back to DVE for the exp-heavy late chunks.
  - normalization is per-(t,h), fully pipelined: denominator row 64 of the
    attnV PSUM -> [1,CH] SBUF copy (DVE) -> reciprocal_approx_fast (DVE)
    -> DRAM-bounced broadcast DMA -> one DVE multiply per head-half.
    No cross-t barrier.  The final (j,t) instead broadcasts its reciprocal
    rows across partitions with two K=1 f32 PE matmuls (no DMA latency),
    and chunk 3's out-proj t<3 accumulation is emitted ahead of that chain
    (4 PSUM accumulators: 2 from the mm pool, 2 from the idle attnV ring)
    so the PE streams straight through the reciprocal resolution.
  - out-proj emits out^T [n, s] in bf16; the host transposes back.
  - Tile builds STATIC per-engine instruction streams.  Scheduling:
    chunk j's attention kt-loop interleaves chunk j+1's QKV projection
    (chunks 0-2 are PE-bound with just that); all three earlier chunks'
    out-projections are deferred into chunk 3's kt-loop, which otherwise
    starves the PE (no next-chunk QKV and exp-heavy ScalarE).
  - DMA queues: x loads + wk on Sync (x issued a full chunk early), wq,
    wv, wo on ScalarE, out^T stores alternating gpsimd/Sync.  The startup
    loads are split into quarters/halves and issued in deadline order so
    the first projection's operands get the full HBM bandwidth.
All matmul inputs are bf16 (1 cycle/row on the PE; fp32r is a 2-pass
format at ~2 cycles/row); accumulation stays fp32 in PSUM.
"""

import numpy as np

B, S, D = 4, 2048, 1024
H, DH = 16, 64
HPC = 8            # heads per core
M = HPC * DH       # 512: per-core qkv out dim / wo in dim
NCORE = 8
CH = 512           # q/s chunk size
NCH = S // CH      # 4
ND = D // 128      # 8  d-tiles (contraction for qkv proj)
NMT = M // 128     # 4  m-tiles (= head pairs)
NKT = S // 128     # 16 k-tiles
NNT = D // 128     # 8  n-tiles (out proj)

LAST_RESULT = None  # BassKernelResults of the most recent run (for test.py)


def _emit(nc, tc, tile, mybir, aps):
    import concourse.bass as bass  # noqa: F401

    f32 = mybir.dt.float32
    bf16 = mybir.dt.bfloat16
    EXP = mybir.ActivationFunctionType.Exp
    xT, wq, wk, wv, wo, outT = aps

    with (
        tc.tile_pool(name="w", bufs=1) as pw,
        tc.tile_pool(name="kv", bufs=1) as pkv,
        tc.tile_pool(name="qt", bufs=2) as pq,
        tc.tile_pool(name="ct", bufs=4) as pct,
        tc.tile_pool(name="x", bufs=2) as px,
        tc.tile_pool(name="u", bufs=6) as pu,
        tc.tile_pool(name="sm", bufs=4) as psm,
        tc.tile_pool(name="o", bufs=4) as po,
        tc.tile_pool(name="cn", bufs=2) as pcn,
        tc.tile_pool(name="dscratch", bufs=4, space="DRAM") as pdram,
        tc.tile_pool(name="ps_mm", bufs=2, space="PSUM") as pp_mm,
        tc.tile_pool(name="ps_sc", bufs=2, space="PSUM") as pp_sc,
        tc.tile_pool(name="ps_av", bufs=2, space="PSUM") as pp_av,
    ):
        xt_map = {}

        def x_load(j, split=False):
            xa = px.tile([128, ND * CH], bf16, name="xa", tag="xa")
            o = ND * CH * j
            if split:
                qw = ND * CH // 4
                for i in range(4):
                    nc.sync.dma_start(
                        out=xa[:, qw * i:qw * (i + 1)],
                        in_=xT[:, o + qw * i:o + qw * (i + 1)],
                    )
            else:
                nc.sync.dma_start(out=xa, in_=xT[:, o:o + ND * CH])
            xt_map[j] = [xa[:, CH * d:CH * (d + 1)] for d in range(ND)]

        # ---- startup loads, deadline-ordered so the first-needed halves get
        # the full HBM bandwidth: sync carries x0 then wk, scalar carries wq
        # then wv then wo; each split in halves matching the d<4 / d>=4 order
        # the projection units consume them in.
        def wtile(nm, cols):
            return pw.tile([128, cols], bf16, name=nm, tag=nm)

        wq_all, wk_all = wtile("wqa", ND * M), wtile("wka", ND * M)
        wv_all, wo_all = wtile("wva", ND * M), wtile("woa", NMT * D)
        hw = ND * M // 2
        qw = ND * M // 4
        x_load(0, split=True)
        for i in range(4):
            nc.scalar.dma_start(
                out=wq_all[:, qw * i:qw * (i + 1)],
                in_=wq[:, qw * i:qw * (i + 1)],
            )
        nc.sync.dma_start(out=wk_all[:, 0:hw], in_=wk[:, 0:hw])
        nc.sync.dma_start(out=wk_all[:, hw:], in_=wk[:, hw:])
        nc.scalar.dma_start(out=wv_all[:, 0:hw], in_=wv[:, 0:hw])
        nc.scalar.dma_start(out=wv_all[:, hw:], in_=wv[:, hw:])
        nc.scalar.dma_start(out=wo_all, in_=wo)
        wq_sb = [wq_all[:, M * d:M * (d + 1)] for d in range(ND)]
        wk_sb = [wk_all[:, M * d:M * (d + 1)] for d in range(ND)]
        wv_sb = [wv_all[:, M * d:M * (d + 1)] for d in range(ND)]
        wo_sb = [wo_all[:, D * t:D * (t + 1)] for t in range(NMT)]

        # head-half selector rows for the PE-broadcast normalize:
        # onesel[h] has ones in columns 64h..64h+63, zeros elsewhere.
        onesel = []
        for h in range(2):
            sel = pw.tile([1, 128], f32, name=f"sel{h}", tag=f"sel{h}")
            nc.gpsimd.memset(sel, 0.0)
            nc.gpsimd.memset(sel[:, 64 * h:64 * (h + 1)], 1.0)
            onesel.append(sel)

        # ---- V storage: [s, 8 heads x (64 V + 1 ones)] ----
        vau = []
        for st in range(NKT):
            v = pkv.tile([128, HPC * 65], bf16, name=f"vau{st}", tag=f"vau{st}")
            nc.gpsimd.memset(
                v.rearrange("p (h c) -> p h c", c=65)[:, :, 64:65], 1.0
            )
            vau.append(v)
        kt_sb = [[None] * NCH for _ in range(NMT)]
        qt_all = {}   # j -> [4 tiles]
        ct_all = {}   # j -> [4 tiles]
        last_box = []  # (cn, c, [r_h0, r_h1]) of the final (j, t)
        upre = {}     # (j, t, kt) -> u tile precomputed a chunk early
        PRE = 3       # leading off-diagonal kt per (j, t) exp'd a chunk early

        # ---------- emission units ----------

        def proj_half(ps, w_sb, t, xt, half, kind):
            """4 of the 8 contraction steps of one projection m-tile."""
            for d in range(4 * half, 4 * half + 4):
                if kind == "v":
                    lhsT = xt[d][:, 128 * t:128 * (t + 1)]
                    rhs = w_sb[d]
                else:
                    lhsT = w_sb[d][:, 128 * t:128 * (t + 1)]
                    rhs = xt[d]
                nc.tensor.matmul(
                    ps, lhsT=lhsT, rhs=rhs,
                    start=(d == 0), stop=(d == ND - 1),
                )

        def qkv_units(j):
            """Generator of emission closures for chunk j's QKV projection."""
            xt = xt_map[j]

            qts = []
            qt_all[j] = qts
            for t in range(NMT):
                ps_box = []

                def qa(t=t, ps_box=ps_box):
                    ps = pp_mm.tile([128, CH], f32, name="psq", tag="mm")
                    ps_box.append(ps)
                    proj_half(ps, wq_sb, t, xt, 0, "q")
                def qb(t=t, ps_box=ps_box):
                    ps = ps_box[0]
                    proj_half(ps, wq_sb, t, xt, 1, "q")
                    q_t = pq.tile([128, CH], bf16, name=f"q{t}", tag=f"q{t}")
                    nc.vector.tensor_copy(out=q_t, in_=ps)
                    qts.append(q_t)
                yield qa
                yield qb
            for t in range(NMT):
                ps_box = []

                def ka(t=t, ps_box=ps_box):
                    ps = pp_mm.tile([128, CH], f32, name="psk", tag="mm")
                    ps_box.append(ps)
                    proj_half(ps, wk_sb, t, xt, 0, "k")
                def kb(t=t, ps_box=ps_box, j=j):
                    ps = ps_box[0]
                    proj_half(ps, wk_sb, t, xt, 1, "k")
                    k_t = pkv.tile(
                        [128, CH], bf16, name=f"k{t}_{j}", tag=f"k{t}_{j}"
                    )
                    nc.vector.tensor_copy(out=k_t, in_=ps)
                    kt_sb[t][j] = k_t
                yield ka
                yield kb
            for st in range(NMT):
                ps_box = []

                def va(st=st, ps_box=ps_box):
                    ps = pp_mm.tile([128, M], f32, name="psv", tag="mm")
                    ps_box.append(ps)
                    proj_half(ps, wv_sb, st, xt, 0, "v")
                def vb(st=st, ps_box=ps_box, j=j):
                    ps = ps_box[0]
                    proj_half(ps, wv_sb, st, xt, 1, "v")
                    g = vau[4 * j + st]
                    nc.vector.tensor_copy(
                        out=g.rearrange("p (h c) -> p h c", c=65)[:, :, 0:64],
                        in_=ps.rearrange("p (h c) -> p h c", c=64),
                    )
                yield va
                yield vb

        def pre_exp_units(j):
            """Chunk j's leading off-diagonal scores+exp, emitted as fillers
            during chunk j-1 (after its QKV units).  Shifts ScalarE exp load
            one chunk earlier: the last chunk is exp-bound, the first has
            ScalarE slack.  kt < PRE <= 4 so these rows are never masked and
            q/k come from chunks that are already resident."""
            for t in range(NMT):
                for kt in range(PRE):
                    def pe_(t=t, kt=kt, j=j):
                        ck, ks = kt // 4, (kt % 4) * 128
                        sc = pp_sc.tile([128, 2 * CH], f32, name="sc", tag="sc")
                        for h in range(2):
                            pb = 64 * h
                            nc.tensor.matmul(
                                sc[:, CH * h:CH * (h + 1)],
                                lhsT=kt_sb[t][ck][pb:pb + 64, ks:ks + 128],
                                rhs=qt_all[j][t][pb:pb + 64, :],
                                start=True,
                                stop=True,
                                tile_position=(pb, 0),
                            )
                        u = pu.tile(
                            [128, 2 * CH], bf16, name="upre", tag="upre",
                            bufs=NMT * PRE,
                        )
                        nc.scalar.activation(out=u, in_=sc, func=EXP, scale=0.125)
                        upre[(j, t, kt)] = u
                    yield pe_

        def outproj_units(j):
            """Generator of emission closures for chunk j's out-projection."""
            for nt in range(NNT):
                def og(nt=nt, j=j):
                    ct = ct_all[j]
                    ps = pp_mm.tile([128, CH], f32, name="pso", tag="mm")
                    for t in range(NMT):
                        nc.tensor.matmul(
                            ps,
                            lhsT=wo_sb[t][:, 128 * nt:128 * (nt + 1)],
                            rhs=ct[t],
                            start=(t == 0),
                            stop=(t == NMT - 1),
                        )
                    o_sb = po.tile([128, CH], bf16, name="osb", tag="o")
                    nc.vector.tensor_copy(out=o_sb, in_=ps)
                    nc.gpsimd.dma_start(
                        out=outT[128 * nt:128 * (nt + 1), CH * j:CH * (j + 1)],
                        in_=o_sb,
                    )
                yield og

        # ---------- chunk 0 QKV up front ----------
        for unit in qkv_units(0):
            unit()

        # ---------- main loop: attention(j) with interleaved fillers ----------
        for j in range(NCH):
            if j + 1 < NCH:
                x_load(j + 1)  # issue the DMA a full chunk early
            fillers = []
            if j + 1 < NCH:
                fillers.extend(qkv_units(j + 1))
                fillers.extend(pre_exp_units(j + 1))
            else:
                for jj in range(NCH - 1):
                    fillers.extend(outproj_units(jj))
            nkt = 4 * (j + 1)
            n_units = NMT * (nkt + 4)
            # in the last chunk, hold a few fillers back past the t-loop so
            # their PSUM drains don't sit on the DVE queue ahead of the final
            # head-pair's normalization chain (and so they cover it instead)
            n_fill = len(fillers) - (3 if j == NCH - 1 else 0)
            popped = 0
            ucount = 0

            qt = qt_all[j]
            ct = []
            ct_all[j] = ct
            for t in range(NMT):
                av = [
                    pp_av.tile([65, CH], f32, name=f"av{h}", tag="av")
                    for h in range(2)
                ]
                us = {}
                for kt in range(nkt + 4):
                    if kt < nkt and (j, t, kt) in upre:
                        us[kt] = (upre.pop((j, t, kt)), 0, CH)
                    elif kt < nkt:
                        dd = kt - 4 * j      # diagonal index (>=0 on diag)
                        qoff = 128 * dd if dd >= 0 else 0
                        n = CH - qoff
                        ck, ks = kt // 4, (kt % 4) * 128
                        # both heads' scores in one 2-bank PSUM tile
                        sc = pp_sc.tile([128, 2 * CH], f32, name="sc", tag="sc")
                        for h in range(2):
                            pb = 64 * h
                            nc.tensor.matmul(
                                sc[:, CH * h:CH * h + n],
                                lhsT=kt_sb[t][ck][pb:pb + 64, ks:ks + 128],
                                rhs=qt[t][pb:pb + 64, qoff:CH],
                                start=True,
                                stop=True,
                                tile_position=(pb, 0),
                            )
                        u = pu.tile([128, 2 * CH], bf16, name="u", tag="u")
                        scv = sc.rearrange("p (h q) -> p h q", h=2)[:, :, 0:n]
                        uv = u.rearrange("p (h q) -> p h q", h=2)[:, :, 0:n]
                        nc.scalar.activation(out=uv, in_=scv, func=EXP, scale=0.125)
                        if dd >= 0:
                            # keep where q_rel >= k_partition (same mask, both)
                            nc.gpsimd.affine_select(
                                out=uv,
                                in_=uv,
                                compare_op=mybir.AluOpType.is_ge,
                                fill=0.0,
                                base=0,
                                channel_multiplier=-1,
                                pattern=[[0, 2], [1, n]],
                            )
                        us[kt] = (u, qoff, n)
                    if kt >= 4:
                        # attnV trails exp by four kt, so each exp gets the
                        # window of four scores matmuls plus fillers to
                        # finish before the PE needs its output or PSUM slot
                        pkt = kt - 4
                        u_p, qoff_p, n_p = us.pop(pkt)
                        for h in range(2):
                            ha = 2 * t + h
                            nc.tensor.matmul(
                                av[h][:, qoff_p:CH],
                                lhsT=vau[pkt][:, 65 * ha:65 * ha + 65],
                                rhs=u_p[:, CH * h:CH * h + n_p],
                                start=(pkt == 0),
                                stop=(pkt == nkt - 1),
                            )
                    ucount += 1
                    while fillers and popped < ucount * n_fill // n_units:
                        fillers.pop(0)()
                        popped += 1

                # drain PSUM: unnormalized C (bf16) + per-(t,h) softmax
                # normalization, fully pipelined (no cross-t barrier).  The
                # very last t of the last chunk takes a latency-optimized
                # path: reciprocal rows broadcast across partitions via two
                # K=1 PE matmuls (f32) instead of the DRAM-bounced DMA.
                last = (j == NCH - 1) and (t == NMT - 1)
                cn_t = pcn.tile([128, CH], bf16, name=f"cn{t}", tag=f"cn{t}")
                c_t = pct.tile([128, CH], bf16, name=f"c{t}", tag=f"c{t}")
                # ScalarE has slack in the early chunks (exp load grows with
                # j) while DVE is the binding engine there — shift the
                # denominator-row drain accordingly.
                rhs_ = []
                for h in range(2):
                    dh = psm.tile([1, CH], f32, name="dh", tag="dh")
                    if j <= 1:
                        nc.scalar.copy(out=dh, in_=av[h][64:65, :])
                    else:
                        nc.vector.tensor_copy(out=dh, in_=av[h][64:65, :])
                    rh = psm.tile([1, CH], f32, name="rh", tag="rh")
                    nc.vector.reciprocal_approx_fast(out=rh, in_=dh)
                    rhs_.append(rh)
                if last:
                    for h in range(2):
                        nc.vector.tensor_copy(
                            out=cn_t[64 * h:64 * (h + 1), :], in_=av[h][0:64, :]
                        )
                    last_box.append((cn_t, c_t, rhs_))
                    ct.append(c_t)
                    continue
                bc = pcn.tile([128, CH], f32, name="bc", tag="bc")
                for h in range(2):
                    if j == 0:
                        nc.scalar.copy(
                            out=cn_t[64 * h:64 * (h + 1), :], in_=av[h][0:64, :]
                        )
                    else:
                        nc.vector.tensor_copy(
                            out=cn_t[64 * h:64 * (h + 1), :], in_=av[h][0:64, :]
                        )
                    rd = pdram.tile([1, CH], f32, name="rd", tag="rd")
                    nc.sync.dma_start(out=rd, in_=rhs_[h])
                    nc.sync.dma_start(
                        out=bc[64 * h:64 * (h + 1), :],
                        in_=rd.to_broadcast((64, CH)),
                    )
                for h in range(2):
                    nc.vector.tensor_mul(
                        c_t[64 * h:64 * (h + 1), :],
                        cn_t[64 * h:64 * (h + 1), :],
                        bc[64 * h:64 * (h + 1), :],
                    )
                ct.append(c_t)

            # leftover fillers for this round
            for f in fillers:
                f()

        # ---- tail: chunk 3's out-projection, with its t<3 accumulation
        # emitted ahead of the last head-pair's normalize so the PE keeps
        # streaming while the reciprocal chain resolves.
        jf = NCH - 1
        ctf = ct_all[jf]
        boxes = {}

        def og_partial(nt, pool):
            ps = pool.tile([128, CH], f32, name="pso", tag=pool.name[3:])
            boxes[nt] = ps
            for t in range(NMT - 1):
                nc.tensor.matmul(
                    ps,
                    lhsT=wo_sb[t][:, 128 * nt:128 * (nt + 1)],
                    rhs=ctf[t],
                    start=(t == 0),
                    stop=False,
                )

        def og_final(nt):
            ps = boxes[nt]
            nc.tensor.matmul(
                ps,
                lhsT=wo_sb[NMT - 1][:, 128 * nt:128 * (nt + 1)],
                rhs=ctf[NMT - 1],
                start=False,
                stop=True,
            )
            o_sb = po.tile([128, CH], bf16, name="osb", tag="o")
            if nt % 2 == 0:
                nc.vector.tensor_copy(out=o_sb, in_=ps)
            else:
                nc.scalar.copy(out=o_sb, in_=ps)
            eng = nc.gpsimd if nt % 2 == 0 else nc.sync
            eng.dma_start(
                out=outT[128 * nt:128 * (nt + 1), CH * jf:CH * (jf + 1)],
                in_=o_sb,
            )

        og_partial(0, pp_mm)
        og_partial(1, pp_mm)
        og_partial(2, pp_av)
        og_partial(3, pp_av)

        # last (j,t) normalize via PE broadcast: bc[m,q] = r_{h(m)}[q], one
        # bf16 K=1 matmul per head into separate PSUM tiles (from the now
        # idle scores ring; both mm-pool buffers are held by the in-flight
        # og partials) so each head's multiply starts as soon as its own
        # reciprocal lands.
        cn_t, c_t, rhs_ = last_box[0]
        for h in range(2):
            bc_ps = pp_sc.tile([128, 2 * CH], f32, name="bcps", tag="sc")
            nc.tensor.matmul(
                bc_ps[:, 0:CH], lhsT=onesel[h], rhs=rhs_[h],
                start=True, stop=True,
            )
            nc.vector.tensor_mul(
                c_t[64 * h:64 * (h + 1), :],
                cn_t[64 * h:64 * (h + 1), :],
                bc_ps[64 * h:64 * (h + 1), 0:CH],
            )

        for nt in range(4):
            og_final(nt)
        og_partial(4, pp_mm)
        og_partial(5, pp_mm)
        og_partial(6, pp_av)
        og_partial(7, pp_av)
        for nt in range(4, NNT):
            og_final(nt)


_PROG = None


def _build():
    global _PROG
    if _PROG is not None:
        return _PROG
    import concourse.bacc as bacc
    import concourse.mybir as mybir
    import concourse.tile as tile

    bf16 = mybir.dt.bfloat16
    nc = bacc.Bacc(
        "TRN2", target_bir_lowering=False, debug=False, enable_asserts=False
    )
    xT = nc.dram_tensor("xT", [128, NCH * ND * CH], bf16, kind="ExternalInput").ap()
    wq = nc.dram_tensor("wq", [128, ND * M], bf16, kind="ExternalInput").ap()
    wk = nc.dram_tensor("wk", [128, ND * M], bf16, kind="ExternalInput").ap()
    wv = nc.dram_tensor("wv", [128, ND * M], bf16, kind="ExternalInput").ap()
    wo = nc.dram_tensor("wo", [128, NMT * D], bf16, kind="ExternalInput").ap()
    outT = nc.dram_tensor("outT", [D, S], bf16, kind="ExternalOutput").ap()

    with tile.TileContext(nc) as tc:
        _emit(nc, tc, tile, mybir, (xT, wq, wk, wv, wo, outT))
    nc.compile()
    _PROG = nc
    return nc


def kernel(x, Wq, Wk, Wv, Wo, bo):
    global LAST_RESULT
    import os

    from concourse.bass_utils import run_bass_kernel_spmd

    x = np.asarray(x, dtype=np.float32)
    Wq = np.asarray(Wq, dtype=np.float32)
    Wk = np.asarray(Wk, dtype=np.float32)
    Wv = np.asarray(Wv, dtype=np.float32)
    Wo = np.asarray(Wo, dtype=np.float32)
    bo = np.asarray(bo, dtype=np.float32)

    nc = _build()

    import ml_dtypes

    bf = ml_dtypes.bfloat16

    def fold_w(w):
        # [(nd p), c] -> [p, (nd c)]
        ndt = w.shape[0] // 128
        return np.ascontiguousarray(
            w.reshape(ndt, 128, w.shape[1]).transpose(1, 0, 2).reshape(128, -1)
        ).astype(bf)

    in_maps = []
    for c in range(NCORE):
        b, g = c // 2, c % 2
        cols = slice(M * g, M * (g + 1))
        xt = x[b].T  # [D, S]
        # [p, (j d s)]: xf[p, j*ND*CH + d*CH + s] = xT[128d+p, CH*j+s]
        xf = (
            xt.reshape(ND, 128, NCH, CH)
            .transpose(1, 2, 0, 3)
            .reshape(128, NCH * ND * CH)
        )
        in_maps.append(
            {
                "xT": np.ascontiguousarray(xf).astype(bf),
                "wq": fold_w(Wq[:, cols]),
                "wk": fold_w(Wk[:, cols]),
                "wv": fold_w(Wv[:, cols]),
                "wo": fold_w(Wo[cols, :]),
            }
        )

    res = run_bass_kernel_spmd(
        nc,
        in_maps,
        list(range(NCORE)),
        trace=bool(os.environ.get("KERNEL_TRACE")),
        tmpdir=os.environ.get("KERNEL_TRACE_DIR") or None,
    )
    LAST_RESULT = res

    out = np.empty((B, S, D), dtype=np.float32)
    for b in range(B):
        acc = res.results[2 * b]["outT"].astype(np.float32) + res.results[
            2 * b + 1
        ]["outT"].astype(np.float32)
        out[b] = acc.T + bo[None, :]
    return out


# revision 51
# speedup vs baseline: 1.0071x; 1.0071x over previous
"""Multi-head causal attention (B=4, S=2048, D=1024, H=16) on 8 TRN2 cores.

Sharding: core c -> batch c//2, head-group c%2 (8 heads, 512 of the 1024
QKV columns / Wo rows).  Each core runs a fused QKV->attention->out-proj
kernel on its shard; the host sums the two head-group partials per batch.

Per-core layout choices:
  - x is fed pre-transposed (xT [D, S]) so Q^T/K^T come out of the PE in
    [m, s] layout and V in natural [s, m] layout with no on-chip transposes.
  - scores are computed transposed (S^T [k, q]); softmax runs as
    exp (ScalarE, scale=1/8 fused, both heads of a pair in one op) ->
    causal mask (gpsimd affine_select, fill=0, diagonal tiles only,
    fully-masked q-ranges skipped entirely) -> attnV matmul with a
    ones-column appended to V (M=65) so the softmax denominator
    accumulates for free in PSUM row 64.
  - attnV trails exp by THREE kt, so each exp has the window of three
    scores matmuls plus fillers before the PE needs its output or PSUM slot.
  - exp waterfall: each chunk's first PRE off-diagonal (t, kt) tiles are
    scored+exp'd a chunk EARLY (as fillers, after that chunk's QKV) with
    the u tiles parked in SBUF — the last chunk is exp-bound on ScalarE
    while the first has ScalarE slack.  The early-chunk denominator-row
    and cn drains likewise run on ScalarE (DVE is the binding engine
    there), switching back to DVE for the exp-heavy late chunks.
  - normalization is per-(t,h), fully pipelined: denominator row 64 of the
    attnV PSUM -> [1,CH] SBUF copy (DVE) -> reciprocal_approx_fast (DVE)
    -> DRAM-bounced broadcast DMA -> one DVE multiply per head-half.
    No cross-t barrier.  The final (j,t) instead broadcasts its reciprocal
    rows across partitions with two K=1 f32 PE matmuls (no DMA latency),
    and chunk 3's out-proj t<3 accumulation is emitted ahead of that chain
    (4 PSUM accumulators: 2 from the mm pool, 2 from the idle attnV ring)
    so the PE streams straight through the reciprocal resolution.
  - out-proj emits out^T [n, s] in bf16; the host transposes back.
  - Tile builds STATIC per-engine instruction streams.  Scheduling:
    chunk j's attention kt-loop interleaves chunk j+1's QKV projection
    (chunks 0-2 are PE-bound with just that); all three earlier chunks'
    out-projections are deferred into chunk 3's kt-loop, which otherwise
    starves the PE (no next-chunk QKV and exp-heavy ScalarE).
  - DMA queues: x loads + wk on Sync (x issued a full chunk early), wq,
    wv, wo on ScalarE, out^T stores alternating gpsimd/Sync.  The startup
    loads are split into quarters/halves and issued in deadline order so
    the first projection's operands get the full HBM bandwidth.
All matmul inputs are bf16 (1 cycle/row on the PE; fp32r is a 2-pass
format at ~2 cycles/row); accumulation stays fp32 in PSUM.
"""

import numpy as np

B, S, D = 4, 2048, 1024
H, DH = 16, 64
HPC = 8            # heads per core
M = HPC * DH       # 512: per-core qkv out dim / wo in dim
NCORE = 8
CH = 512           # q/s chunk size
NCH = S // CH      # 4
ND = D // 128      # 8  d-tiles (contraction for qkv proj)
NMT = M // 128     # 4  m-tiles (= head pairs)
NKT = S // 128     # 16 k-tiles
NNT = D // 128     # 8  n-tiles (out proj)

LAST_RESULT = None  # BassKernelResults of the most recent run (for test.py)


def _emit(nc, tc, tile, mybir, aps):
    import concourse.bass as bass  # noqa: F401

    f32 = mybir.dt.float32
    bf16 = mybir.dt.bfloat16
    EXP = mybir.ActivationFunctionType.Exp
    xT, wq, wk, wv, wo, outT = aps

    with (
        tc.tile_pool(name="w", bufs=1) as pw,
        tc.tile_pool(name="kv", bufs=1) as pkv,
        tc.tile_pool(name="qt", bufs=2) as pq,
        tc.tile_pool(name="ct", bufs=4) as pct,
        tc.tile_pool(name="x", bufs=2) as px,
        tc.tile_pool(name="u", bufs=6) as pu,
        tc.tile_pool(name="sm", bufs=4) as psm,
        tc.tile_pool(name="o", bufs=4) as po,
        tc.tile_pool(name="cn", bufs=2) as pcn,
        tc.tile_pool(name="dscratch", bufs=4, space="DRAM") as pdram,
        tc.tile_pool(name="ps_mm", bufs=2, space="PSUM") as pp_mm,
        tc.tile_pool(name="ps_sc", bufs=2, space="PSUM") as pp_sc,
        tc.tile_pool(name="ps_av", bufs=2, space="PSUM") as pp_av,
    ):
        xt_map = {}

        def x_load(j, split=False):
            xa = px.tile([128, ND * CH], bf16, name="xa", tag="xa")
            o = ND * CH * j
            if split:
                qw = ND * CH // 4
                for i in range(4):
                    nc.sync.dma_start(
                        out=xa[:, qw * i:qw * (i + 1)],
                        in_=xT[:, o + qw * i:o + qw * (i + 1)],
                    )
            else:
                nc.sync.dma_start(out=xa, in_=xT[:, o:o + ND * CH])
            xt_map[j] = [xa[:, CH * d:CH * (d + 1)] for d in range(ND)]

        # ---- startup loads, deadline-ordered so the first-needed halves get
        # the full HBM bandwidth: sync carries x0 then wk, scalar carries wq
        # then wv then wo; each split in halves matching the d<4 / d>=4 order
        # the projection units consume them in.
        def wtile(nm, cols):
            return pw.tile([128, cols], bf16, name=nm, tag=nm)

        wq_all, wk_all = wtile("wqa", ND * M), wtile("wka", ND * M)
        wv_all, wo_all = wtile("wva", ND * M), wtile("woa", NMT * D)
        hw = ND * M // 2
        qw = ND * M // 4
        x_load(0, split=True)
        for i in range(4):
            nc.scalar.dma_start(
                out=wq_all[:, qw * i:qw * (i + 1)],
                in_=wq[:, qw * i:qw * (i + 1)],
            )
        nc.sync.dma_start(out=wk_all[:, 0:hw], in_=wk[:, 0:hw])
        nc.sync.dma_start(out=wk_all[:, hw:], in_=wk[:, hw:])
        nc.scalar.dma_start(out=wv_all[:, 0:hw], in_=wv[:, 0:hw])
        nc.scalar.dma_start(out=wv_all[:, hw:], in_=wv[:, hw:])
        nc.scalar.dma_start(out=wo_all, in_=wo)
        wq_sb = [wq_all[:, M * d:M * (d + 1)] for d in range(ND)]
        wk_sb = [wk_all[:, M * d:M * (d + 1)] for d in range(ND)]
        wv_sb = [wv_all[:, M * d:M * (d + 1)] for d in range(ND)]
        wo_sb = [wo_all[:, D * t:D * (t + 1)] for t in range(NMT)]

        # head-half selector rows for the PE-broadcast normalize:
        # onesel[h] has ones in columns 64h..64h+63, zeros elsewhere.
        onesel = []
        for h in range(2):
            sel = pw.tile([1, 128], f32, name=f"sel{h}", tag=f"sel{h}")
            nc.gpsimd.memset(sel, 0.0)
            nc.gpsimd.memset(sel[:, 64 * h:64 * (h + 1)], 1.0)
            onesel.append(sel)

        # ---- V storage: [s, 8 heads x (64 V + 1 ones)] ----
        vau = []
        for st in range(NKT):
            v = pkv.tile([128, HPC * 65], bf16, name=f"vau{st}", tag=f"vau{st}")
            nc.gpsimd.memset(
                v.rearrange("p (h c) -> p h c", c=65)[:, :, 64:65], 1.0
            )
            vau.append(v)
        kt_sb = [[None] * NCH for _ in range(NMT)]
        qt_all = {}   # j -> [4 tiles]
        ct_all = {}   # j -> [4 tiles]
        last_box = []  # (cn, c, [r_h0, r_h1]) of the final (j, t)
        upre = {}     # (j, t, kt) -> u tile precomputed a chunk early
        PRE = 3       # leading off-diagonal kt per (j, t) exp'd a chunk early

        # ---------- emission units ----------

        def proj_half(ps, w_sb, t, xt, half, kind):
            """4 of the 8 contraction steps of one projection m-tile."""
            for d in range(4 * half, 4 * half + 4):
                if kind == "v":
                    lhsT = xt[d][:, 128 * t:128 * (t + 1)]
                    rhs = w_sb[d]
                else:
                    lhsT = w_sb[d][:, 128 * t:128 * (t + 1)]
                    rhs = xt[d]
                nc.tensor.matmul(
                    ps, lhsT=lhsT, rhs=rhs,
                    start=(d == 0), stop=(d == ND - 1),
                )

        def qkv_units(j):
            """Generator of emission closures for chunk j's QKV projection."""
            xt = xt_map[j]

            qts = []
            qt_all[j] = qts
            for t in range(NMT):
                ps_box = []

                def qa(t=t, ps_box=ps_box):
                    ps = pp_mm.tile([128, CH], f32, name="psq", tag="mm")
                    ps_box.append(ps)
                    proj_half(ps, wq_sb, t, xt, 0, "q")
                def qb(t=t, ps_box=ps_box):
                    ps = ps_box[0]
                    proj_half(ps, wq_sb, t, xt, 1, "q")
                    q_t = pq.tile([128, CH], bf16, name=f"q{t}", tag=f"q{t}")
                    nc.vector.tensor_copy(out=q_t, in_=ps)
                    qts.append(q_t)
                yield qa
                yield qb
            for t in range(NMT):
                ps_box = []

                def ka(t=t, ps_box=ps_box):
                    ps = pp_mm.tile([128, CH], f32, name="psk", tag="mm")
                    ps_box.append(ps)
                    proj_half(ps, wk_sb, t, xt, 0, "k")
                def kb(t=t, ps_box=ps_box, j=j):
                    ps = ps_box[0]
                    proj_half(ps, wk_sb, t, xt, 1, "k")
                    k_t = pkv.tile(
                        [128, CH], bf16, name=f"k{t}_{j}", tag=f"k{t}_{j}"
                    )
                    nc.vector.tensor_copy(out=k_t, in_=ps)
                    kt_sb[t][j] = k_t
                yield ka
                yield kb
            for st in range(NMT):
                ps_box = []

                def va(st=st, ps_box=ps_box):
                    ps = pp_mm.tile([128, M], f32, name="psv", tag="mm")
                    ps_box.append(ps)
                    proj_half(ps, wv_sb, st, xt, 0, "v")
                def vb(st=st, ps_box=ps_box, j=j):
                    ps = ps_box[0]
                    proj_half(ps, wv_sb, st, xt, 1, "v")
                    g = vau[4 * j + st]
                    nc.vector.tensor_copy(
                        out=g.rearrange("p (h c) -> p h c", c=65)[:, :, 0:64],
                        in_=ps.rearrange("p (h c) -> p h c", c=64),
                    )
                yield va
                yield vb

        def pre_exp_units(j):
            """Chunk j's leading off-diagonal scores+exp, emitted as fillers
            during chunk j-1 (after its QKV units).  Shifts ScalarE exp load
            one chunk earlier: the last chunk is exp-bound, the first has
            ScalarE slack.  kt < PRE <= 4 so these rows are never masked and
            q/k come from chunks that are already resident."""
            for t in range(NMT):
                for kt in range(PRE):
                    def pe_(t=t, kt=kt, j=j):
                        ck, ks = kt // 4, (kt % 4) * 128
                        sc = pp_sc.tile([128, 2 * CH], f32, name="sc", tag="sc")
                        for h in range(2):
                            pb = 64 * h
                            nc.tensor.matmul(
                                sc[:, CH * h:CH * (h + 1)],
                                lhsT=kt_sb[t][ck][pb:pb + 64, ks:ks + 128],
                                rhs=qt_all[j][t][pb:pb + 64, :],
                                start=True,
                                stop=True,
                                tile_position=(pb, 0),
                            )
                        u = pu.tile(
                            [128, 2 * CH], bf16, name="upre", tag="upre",
                            bufs=NMT * PRE,
                        )
                        nc.scalar.activation(out=u, in_=sc, func=EXP, scale=0.125)
                        upre[(j, t, kt)] = u
                    yield pe_

        def outproj_units(j):
            """Generator of emission closures for chunk j's out-projection."""
            for nt in range(NNT):
                def og(nt=nt, j=j):
                    ct = ct_all[j]
                    ps = pp_mm.tile([128, CH], f32, name="pso", tag="mm")
                    for t in range(NMT):
                        nc.tensor.matmul(
                            ps,
                            lhsT=wo_sb[t][:, 128 * nt:128 * (nt + 1)],
                            rhs=ct[t],
                            start=(t == 0),
                            stop=(t == NMT - 1),
                        )
                    o_sb = po.tile([128, CH], bf16, name="osb", tag="o")
                    nc.vector.tensor_copy(out=o_sb, in_=ps)
                    nc.gpsimd.dma_start(
                        out=outT[128 * nt:128 * (nt + 1), CH * j:CH * (j + 1)],
                        in_=o_sb,
                    )
                yield og

        # ---------- chunk 0 QKV up front ----------
        for unit in qkv_units(0):
            unit()

        # ---------- main loop: attention(j) with interleaved fillers ----------
        for j in range(NCH):
            if j + 1 < NCH:
                x_load(j + 1)  # issue the DMA a full chunk early
            fillers = []
            if j + 1 < NCH:
                fillers.extend(qkv_units(j + 1))
                fillers.extend(pre_exp_units(j + 1))
            else:
                for jj in range(NCH - 1):
                    fillers.extend(outproj_units(jj))
            nkt = 4 * (j + 1)
            n_units = NMT * (nkt + 3)
            # in the last chunk, hold a few fillers back past the t-loop so
            # their PSUM drains don't sit on the DVE queue ahead of the final
            # head-pair's normalization chain (and so they cover it instead)
            n_fill = len(fillers) - (3 if j == NCH - 1 else 0)
            popped = 0
            ucount = 0

            qt = qt_all[j]
            ct = []
            ct_all[j] = ct
            for t in range(NMT):
                av = [
                    pp_av.tile([65, CH], f32, name=f"av{h}", tag="av")
                    for h in range(2)
                ]
                us = {}
                for kt in range(nkt + 3):
                    if kt < nkt and (j, t, kt) in upre:
                        us[kt] = (upre.pop((j, t, kt)), 0, CH)
                    elif kt < nkt:
                        dd = kt - 4 * j      # diagonal index (>=0 on diag)
                        qoff = 128 * dd if dd >= 0 else 0
                        n = CH - qoff
                        ck, ks = kt // 4, (kt % 4) * 128
                        # both heads' scores in one 2-bank PSUM tile
                        sc = pp_sc.tile([128, 2 * CH], f32, name="sc", tag="sc")
                        for h in range(2):
                            pb = 64 * h
                            nc.tensor.matmul(
                                sc[:, CH * h:CH * h + n],
                                lhsT=kt_sb[t][ck][pb:pb + 64, ks:ks + 128],
                                rhs=qt[t][pb:pb + 64, qoff:CH],
                                start=True,
                                stop=True,
                                tile_position=(pb, 0),
                            )
                        u = pu.tile([128, 2 * CH], bf16, name="u", tag="u")
                        scv = sc.rearrange("p (h q) -> p h q", h=2)[:, :, 0:n]
                        uv = u.rearrange("p (h q) -> p h q", h=2)[:, :, 0:n]
                        nc.scalar.activation(out=uv, in_=scv, func=EXP, scale=0.125)
                        if dd >= 0:
                            # keep where q_rel >= k_partition (same mask, both)
                            nc.gpsimd.affine_select(
                                out=uv,
                                in_=uv,
                                compare_op=mybir.AluOpType.is_ge,
                                fill=0.0,
                                base=0,
                                channel_multiplier=-1,
                                pattern=[[0, 2], [1, n]],
                            )
                        us[kt] = (u, qoff, n)
                    if kt >= 3:
                        # attnV trails exp by three kt, so each exp gets the
                        # window of three scores matmuls plus fillers to
                        # finish before the PE needs its output or PSUM slot
                        pkt = kt - 3
                        u_p, qoff_p, n_p = us.pop(pkt)
                        for h in range(2):
                            ha = 2 * t + h
                            nc.tensor.matmul(
                                av[h][:, qoff_p:CH],
                                lhsT=vau[pkt][:, 65 * ha:65 * ha + 65],
                                rhs=u_p[:, CH * h:CH * h + n_p],
                                start=(pkt == 0),
                                stop=(pkt == nkt - 1),
                            )
                    ucount += 1
                    while fillers and popped < ucount * n_fill // n_units:
                        fillers.pop(0)()
                        popped += 1

                # drain PSUM: unnormalized C (bf16) + per-(t,h) softmax
                # normalization, fully pipelined (no cross-t barrier).  The
                # very last t of the last chunk takes a latency-optimized
                # path: reciprocal rows broadcast across partitions via two
                # K=1 PE matmuls (f32) instead of the DRAM-bounced DMA.
                last = (j == NCH - 1) and (t == NMT - 1)
                cn_t = pcn.tile([128, CH], bf16, name=f"cn{t}", tag=f"cn{t}")
                c_t = pct.tile([128, CH], bf16, name=f"c{t}", tag=f"c{t}")
                # ScalarE has slack in the early chunks (exp load grows with
                # j) while DVE is the binding engine there — shift the
                # denominator-row drain accordingly.
                rhs_ = []
                for h in range(2):
                    dh = psm.tile([1, CH], f32, name="dh", tag="dh")
                    if j <= 1:
                        nc.scalar.copy(out=dh, in_=av[h][64:65, :])
                    else:
                        nc.vector.tensor_copy(out=dh, in_=av[h][64:65, :])
                    rh = psm.tile([1, CH], f32, name="rh", tag="rh")
                    nc.vector.reciprocal_approx_fast(out=rh, in_=dh)
                    rhs_.append(rh)
                if last:
                    for h in range(2):
                        nc.vector.tensor_copy(
                            out=cn_t[64 * h:64 * (h + 1), :], in_=av[h][0:64, :]
                        )
                    last_box.append((cn_t, c_t, rhs_))
                    ct.append(c_t)
                    continue
                bc = pcn.tile([128, CH], f32, name="bc", tag="bc")
                for h in range(2):
                    if j == 0:
                        nc.scalar.copy(
                            out=cn_t[64 * h:64 * (h + 1), :], in_=av[h][0:64, :]
                        )
                    else:
                        nc.vector.tensor_copy(
                            out=cn_t[64 * h:64 * (h + 1), :], in_=av[h][0:64, :]
                        )
                    rd = pdram.tile([1, CH], f32, name="rd", tag="rd")
                    nc.sync.dma_start(out=rd, in_=rhs_[h])
                    nc.sync.dma_start(
                        out=bc[64 * h:64 * (h + 1), :],
                        in_=rd.to_broadcast((64, CH)),
                    )
                for h in range(2):
                    nc.vector.tensor_mul(
                        c_t[64 * h:64 * (h + 1), :],
                        cn_t[64 * h:64 * (h + 1), :],
                        bc[64 * h:64 * (h + 1), :],
                    )
                ct.append(c_t)

            # leftover fillers for this round
            for f in fillers:
                f()

        # ---- tail: chunk 3's out-projection, with its t<3 accumulation
        # emitted ahead of the last head-pair's normalize so the PE keeps
        # streaming while the reciprocal chain resolves.
        jf = NCH - 1
        ctf = ct_all[jf]
        boxes = {}

        def og_partial(nt, pool):
            ps = pool.tile([128, CH], f32, name="pso", tag=pool.name[3:])
            boxes[nt] = ps
            for t in range(NMT - 1):
                nc.tensor.matmul(
                    ps,
                    lhsT=wo_sb[t][:, 128 * nt:128 * (nt + 1)],
                    rhs=ctf[t],
                    start=(t == 0),
                    stop=False,
                )

        def og_final(nt):
            ps = boxes[nt]
            nc.tensor.matmul(
                ps,
                lhsT=wo_sb[NMT - 1][:, 128 * nt:128 * (nt + 1)],
                rhs=ctf[NMT - 1],
                start=False,
                stop=True,
            )
            o_sb = po.tile([128, CH], bf16, name="osb", tag="o")
            if nt % 2 == 0:
                nc.vector.tensor_copy(out=o_sb, in_=ps)
            else:
                nc.scalar.copy(out=o_sb, in_=ps)
            eng = nc.gpsimd if nt % 2 == 0 else nc.sync
            eng.dma_start(
                out=outT[128 * nt:128 * (nt + 1), CH * jf:CH * (jf + 1)],
                in_=o_sb,
            )

        og_partial(0, pp_mm)
        og_partial(1, pp_mm)
        og_partial(2, pp_av)
        og_partial(3, pp_av)

        # last (j,t) normalize via PE broadcast: bc[m,q] = r_{h(m)}[q], one
        # bf16 K=1 matmul per head into separate PSUM tiles (from the now
        # idle scores ring; both mm-pool buffers are held by the in-flight
        # og partials) so each head's multiply starts as soon as its own
        # reciprocal lands.
        cn_t, c_t, rhs_ = last_box[0]
        for h in range(2):
            bc_ps = pp_sc.tile([128, 2 * CH], f32, name="bcps", tag="sc")
            nc.tensor.matmul(
                bc_ps[:, 0:CH], lhsT=onesel[h], rhs=rhs_[h],
                start=True, stop=True,
            )
            nc.vector.tensor_mul(
                c_t[64 * h:64 * (h + 1), :],
                cn_t[64 * h:64 * (h + 1), :],
                bc_ps[64 * h:64 * (h + 1), 0:CH],
            )

        for nt in range(4):
            og_final(nt)
        og_partial(4, pp_mm)
        og_partial(5, pp_mm)
        og_partial(6, pp_av)
        og_partial(7, pp_av)
        for nt in range(4, NNT):
            og_final(nt)


_PROG = None


def _build():
    global _PROG
    if _PROG is not None:
        return _PROG
    import concourse.bacc as bacc
    import concourse.mybir as mybir
    import concourse.tile as tile

    bf16 = mybir.dt.bfloat16
    nc = bacc.Bacc(
        "TRN2", target_bir_lowering=False, debug=False, enable_asserts=False
    )
    xT = nc.dram_tensor("xT", [128, NCH * ND * CH], bf16, kind="ExternalInput").ap()
    wq = nc.dram_tensor("wq", [128, ND * M], bf16, kind="ExternalInput").ap()
    wk = nc.dram_tensor("wk", [128, ND * M], bf16, kind="ExternalInput").ap()
    wv = nc.dram_tensor("wv", [128, ND * M], bf16, kind="ExternalInput").ap()
    wo = nc.dram_tensor("wo", [128, NMT * D], bf16, kind="ExternalInput").ap()
    outT = nc.dram_tensor("outT", [D, S], bf16, kind="ExternalOutput").ap()

    with tile.TileContext(nc) as tc:
        _emit(nc, tc, tile, mybir, (xT, wq, wk, wv, wo, outT))
    nc.compile()
    _PROG = nc
    return nc


def kernel(x, Wq, Wk, Wv, Wo, bo):
    global LAST_RESULT
    import os

    from concourse.bass_utils import run_bass_kernel_spmd

    x = np.asarray(x, dtype=np.float32)
    Wq = np.asarray(Wq, dtype=np.float32)
    Wk = np.asarray(Wk, dtype=np.float32)
    Wv = np.asarray(Wv, dtype=np.float32)
    Wo = np.asarray(Wo, dtype=np.float32)
    bo = np.asarray(bo, dtype=np.float32)

    nc = _build()

    import ml_dtypes

    bf = ml_dtypes.bfloat16

    def fold_w(w):
        # [(nd p), c] -> [p, (nd c)]
        ndt = w.shape[0] // 128
        return np.ascontiguousarray(
            w.reshape(ndt, 128, w.shape[1]).transpose(1, 0, 2).reshape(128, -1)
        ).astype(bf)

    in_maps = []
    for c in range(NCORE):
        b, g = c // 2, c % 2
        cols = slice(M * g, M * (g + 1))
        xt = x[b].T  # [D, S]
        # [p, (j d s)]: xf[p, j*ND*CH + d*CH + s] = xT[128d+p, CH*j+s]
        xf = (
            xt.reshape(ND, 128, NCH, CH)
            .transpose(1, 2, 0, 3)
            .reshape(128, NCH * ND * CH)
        )
        in_maps.append(
            {
                "xT": np.ascontiguousarray(xf).astype(bf),
                "wq": fold_w(Wq[:, cols]),
                "wk": fold_w(Wk[:, cols]),
                "wv": fold_w(Wv[:, cols]),
                "wo": fold_w(Wo[cols, :]),
            }
        )

    res = run_bass_kernel_spmd(
        nc,
        in_maps,
        list(range(NCORE)),
        trace=bool(os.environ.get("KERNEL_TRACE")),
        tmpdir=os.environ.get("KERNEL_TRACE_DIR") or None,
    )
    LAST_RESULT = res

    out = np.empty((B, S, D), dtype=np.float32)
    for b in range(B):
        acc = res.results[2 * b]["outT"].astype(np.float32) + res.results[
            2 * b + 1
        ]["outT"].astype(np.float32)
        out[b] = acc.T + bo[None, :]
    return out
